# revision 1
# baseline (speedup 1.0000x reference)
"""EMA head kernel for Trainium2 (Bass/Tile), 8 NeuronCores.

Problem: alpha = clip(sigmoid(MLP(feat)), 0.01, 0.99) per (t, b);
         y[0] = r[0]; y[t] = (1-alpha[t])*y[t-1] + alpha[t]*r[t].

Sharding: time dim T=4096 split into 8 slabs of 512 (all B=256 per core).
Each core computes, for its slab, the local affine-scan pieces
    z[t] = A[t]*z[t-1] + Bv[t]   (z[-1] = 0),   A = 1-alpha, Bv = alpha*r
    P[t] = A[t]*P[t-1]           (P[-1] = 1)
and the host stitches slabs with   y = z + P * carry,  carry' = y[-1].
carry_0 = r[0] reproduces y[0] = r[0] exactly: a*r + (1-a)*r = r.

On-chip layout: feat tiles [128 b, 128 f] (contiguous DMA), PE transpose ->
featT [f, b] (PSUM), copy to SBUF, matmul lhsT=featT rhs=W1 -> h [b, 16]
collected 32 t-steps per PSUM bank, then +b1/relu/*W2/reduce on ACT+DVE in
[128, 512] batches -> alpha_pre [128 b, t], sigmoid+clip, and
tensor_tensor_scan along the free (t) dim for z and P.
"""

import numpy as np

T, B, FEAT, HID = 4096, 256, 128, 16
NCORES = 8
TLOC = T // NCORES  # 512
NH = 2              # batch halves of 128
TG = 8              # t-steps per feat dma_start (1 MB)
# engine assignment for the fp32->fp16 cast of each feat group
CAST_PATTERN = ["g", "g", "v", "g", "g", "s", "g", "v"]

_CACHE = {}


def _build_program():
    import concourse.bacc as bacc
    import concourse.bass as bass
    import concourse.tile as tile
    from concourse import mybir
    from concourse.masks import make_identity

    fp32 = mybir.dt.float32
    fp16 = mybir.dt.float16
    AF = mybir.ActivationFunctionType
    OP = mybir.AluOpType

    nc = bacc.Bacc("TRN2", target_bir_lowering=False, debug=False,
                   num_devices=NCORES)

    feat_d = nc.dram_tensor("feat", [TLOC, B, FEAT], fp32, kind="ExternalInput")
    r_d = nc.dram_tensor("r", [TLOC, B], fp32, kind="ExternalInput")
    w1_d = nc.dram_tensor("w1", [FEAT, HID], fp32, kind="ExternalInput")
    b1_d = nc.dram_tensor("b1", [HID], fp32, kind="ExternalInput")
    w2_d = nc.dram_tensor("w2", [HID], fp32, kind="ExternalInput")
    b2_d = nc.dram_tensor("b2", [1], fp32, kind="ExternalInput")
    z_d = nc.dram_tensor("z", [NH, 128, TLOC], fp32, kind="ExternalOutput")
    p_d = nc.dram_tensor("p", [NH, 128, TLOC], fp32, kind="ExternalOutput")

    with tile.TileContext(nc) as tc:
        with (
            tc.tile_pool(name="singles", bufs=1) as singles,
            tc.tile_pool(name="featin", bufs=3) as featin,
            tc.tile_pool(name="ftps", bufs=3, space="PSUM") as ftps,
            tc.tile_pool(name="hps", bufs=2, space="PSUM") as hps,
            tc.tile_pool(name="ftsb", bufs=3) as ftsb,
            tc.tile_pool(name="hwork", bufs=2) as hwork,
        ):
            # ---------------- constants ----------------
            ident = singles.tile([128, 128], fp16)
            make_identity(nc, ident)
            ident32 = singles.tile([128, 128], fp32)
            make_identity(nc, ident32)
            w1_sb = singles.tile([128, HID], fp16)
            nc.gpsimd.dma_start(w1_sb, w1_d[:, :])
            b1rep = singles.tile([128, 32, HID], fp32)
            nc.gpsimd.dma_start(
                b1rep, bass.AP(b1_d, 0, [[0, 128], [0, 32], [1, HID]]))
            w2rep = singles.tile([128, 32, HID], fp32)
            nc.gpsimd.dma_start(
                w2rep, bass.AP(w2_d, 0, [[0, 128], [0, 32], [1, HID]]))
            b2col = singles.tile([128, 1], fp32)
            nc.gpsimd.dma_start(b2col, bass.AP(b2_d, 0, [[0, 128], [1, 1]]))
            ones_sb = singles.tile([128, TLOC], fp32)
            nc.vector.memset(ones_sb, 1.0)

            # ---- r: load [t, b]; PE-transpose to rT [b, t] per half ----
            rT = [singles.tile([128, TLOC], fp32, tag=f"rT{h}", name=f"rT{h}")
                  for h in range(NH)]
            for tcnk in range(TLOC // 128):
                rload = featin.tile([128, B], fp32, tag="rload")
                nc.sync.dma_start(rload, r_d[tcnk * 128:(tcnk + 1) * 128, :])
                rps = ftps.tile([128, 4, 128], fp32, tag="ftp16")
                rview = rload[:, :].rearrange("p (b j) -> p j b", j=2)
                for h in range(NH):
                    nc.tensor.transpose(rps[:, h, :], rview[:, h, :], ident32)
                for h in range(NH):
                    nc.vector.tensor_copy(
                        rT[h][:, tcnk * 128:(tcnk + 1) * 128], rps[:, h, :])

            # per-half alpha_pre accumulators [128 b, t]
            apre = [singles.tile([128, TLOC], fp32, tag=f"apre{h}", name=f"apre{h}")
                    for h in range(NH)]

            # ---------------- main feat pipeline ----------------
            hbank = [None, None]
            copy_parity = 0
            for t0 in range(0, TLOC, TG):
                if t0 % 32 == 0:
                    hbank[0] = hps.tile([128, 32, HID], fp32, tag="h0", name="hbank0")
                    hbank[1] = hps.tile([128, 32, HID], fp32, tag="h1", name="hbank1")

                fin = featin.tile([128, TG, 2 * FEAT], fp16, tag="fin")
                nc.gpsimd.dma_start(
                    fin,
                    feat_d[t0:t0 + TG, :, :].rearrange(
                        "t (p j) f -> p t (j f)", j=2))

                # groups of 4 tiles: transpose -> psum bank -> copy -> matmul
                for q in range(0, 2 * TG, 4):
                    ftp = ftps.tile([128, 4, 128], fp16, tag="ftp16")
                    for s in range(4):
                        g = q + s
                        tt, j = g // 2, g % 2
                        nc.tensor.transpose(
                            ftp[:, s, :],
                            fin[:, tt, j * FEAT:(j + 1) * FEAT], ident)
                    fts = ftsb.tile([128, 4, 128], fp16, tag="fts")
                    if copy_parity == 0:
                        nc.vector.tensor_copy(fts, ftp)
                    else:
                        nc.scalar.copy(fts, ftp)
                    copy_parity ^= 1
                    for s in range(4):
                        g = q + s
                        tt, h = g // 2, g % 2
                        slot = (t0 + tt) % 32
                        nc.tensor.matmul(hbank[h][:, slot, :],
                                         fts[:, s, :], w1_sb)

                if (t0 + TG) % 32 == 0:
                    blk = t0 // 32
                    for h in range(NH):
                        hb = hwork.tile([128, 32, HID], fp32, tag="hb")
                        nc.vector.tensor_add(hb, hbank[h], b1rep)
                        hrelu = hwork.tile([128, 32, HID], fp32, tag="hrelu")
                        nc.scalar.activation(hrelu, hb, AF.Relu)
                        hw = hwork.tile([128, 32, HID], fp32, tag="hw")
                        nc.vector.tensor_mul(hw, hrelu, w2rep)
                        nc.vector.tensor_reduce(
                            apre[h][:, blk * 32:(blk + 1) * 32],
                            hw, axis=mybir.AxisListType.X, op=OP.add)

            # ---------------- alpha -> scans -> out ----------------
            for h in range(NH):
                alpha = singles.tile([128, TLOC], fp32, tag=f"alpha{h}")
                nc.scalar.activation(alpha, apre[h], AF.Sigmoid, bias=b2col)
                nc.vector.tensor_scalar(alpha, alpha, 0.01, 0.99,
                                        op0=OP.max, op1=OP.min)
                A_sb = singles.tile([128, TLOC], fp32, tag=f"A{h}")
                nc.vector.tensor_scalar(A_sb, alpha, -1.0, 1.0,
                                        op0=OP.mult, op1=OP.add)
                Bv = singles.tile([128, TLOC], fp32, tag=f"Bv{h}")
                nc.vector.tensor_mul(Bv, alpha, rT[h])
                z_sb = singles.tile([128, TLOC], fp32, tag=f"z{h}")
                nc.vector.tensor_tensor_scan(z_sb, A_sb, Bv, 0.0,
                                             op0=OP.mult, op1=OP.add)
                p_sb = singles.tile([128, TLOC], fp32, tag=f"p{h}")
                nc.vector.tensor_tensor_scan(p_sb, A_sb, ones_sb, 1.0,
                                             op0=OP.mult, op1=OP.mult)
                nc.sync.dma_start(z_d[h], z_sb)
                nc.sync.dma_start(p_d[h], p_sb)

    nc.finalize()
    return nc


def _get_program():
    if "nc" not in _CACHE:
        _CACHE["nc"] = _build_program()
    return _CACHE["nc"]


def kernel(r, feat, W1, b1, W2, b2, _run_kwargs=None, _return_results=False):
    from concourse.bass_utils import run_bass_kernel_spmd

    r = np.asarray(r, dtype=np.float32)
    feat = np.asarray(feat, dtype=np.float32)
    W1 = np.asarray(W1, dtype=np.float32)
    b1 = np.asarray(b1, dtype=np.float32).reshape(HID)
    W2 = np.asarray(W2, dtype=np.float32).reshape(HID)
    b2 = np.asarray(b2, dtype=np.float32).reshape(1)

    nc = _get_program()
    in_maps = []
    for c in range(NCORES):
        in_maps.append({
            "feat": np.ascontiguousarray(feat[c * TLOC:(c + 1) * TLOC]),
            "r": np.ascontiguousarray(r[c * TLOC:(c + 1) * TLOC, :, 0]),
            "w1": W1, "b1": b1, "w2": W2, "b2": b2,
        })

    kw = _run_kwargs or {}
    res = run_bass_kernel_spmd(nc, in_maps, core_ids=list(range(NCORES)), **kw)

    # host stitch: y = z + P*carry per slab, carry chain across slabs
    y = np.empty((T, B), dtype=np.float32)
    carry = r[0, :, 0].astype(np.float32)
    for c in range(NCORES):
        zc = res.results[c]["z"].transpose(2, 1, 0).reshape(TLOC, B)
        pc = res.results[c]["p"].transpose(2, 1, 0).reshape(TLOC, B)
        y_slab = zc + pc * carry[None, :]
        carry = y_slab[-1]
        y[c * TLOC:(c + 1) * TLOC] = y_slab
    out = y[:, :, None]
    if _return_results:
        return out, res
    return out



# revision 3
# speedup vs baseline: 1.2432x; 1.2432x over previous
"""EMA head kernel for Trainium2 (Bass/Tile), 8 NeuronCores.

Problem: alpha = clip(sigmoid(MLP(feat)), 0.01, 0.99) per (t, b);
         y[0] = r[0]; y[t] = (1-alpha[t])*y[t-1] + alpha[t]*r[t].

Sharding: time dim T=4096 split into 8 slabs of 512 (all B=256 per core).
Each core computes, for its slab, the local affine-scan pieces
    z[t] = A[t]*z[t-1] + Bv[t]   (z[-1] = 0),   A = 1-alpha, Bv = alpha*r
    P[t] = A[t]*P[t-1]           (P[-1] = 1)
and the host stitches slabs with   y = z + P * carry,  carry' = y[-1].
carry_0 = r[0] reproduces y[0] = r[0] exactly: a*r + (1-a)*r = r.

On-chip layout (v2 — DMA-descriptor-friendly): feat is loaded with the
TIME dim on partitions: tile [t=128, bq=64 * f=128] so each partition
line is one 32 KB contiguous DRAM chunk (line-rate HBM instead of the
1 KB chunks a b-on-partition layout forces).  fp32->fp16 cast happens
in the DMA (SWDGE).  Per batch element: PE transpose [t,f] -> [f,t],
copy PSUM->SBUF, matmul lhsT=ftT rhs=W1 -> h [t, 16] collected 32
b-slots per PSUM bank, then +b1/relu/*W2/reduce -> apre [t=128, b].
apre is PE-transposed back to [b=128, t] for sigmoid/clip and the
tensor_tensor_scan along the free (t) dim for z and P.
"""

import numpy as np

T, B, FEAT, HID = 4096, 256, 128, 16
NCORES = 8
TLOC = T // NCORES  # 512
NH = 2              # batch halves of 128 (contiguous: b = h*128 + p)
NTC = TLOC // 128   # 4 t-chunks of 128 partitions
BQ = 64             # batch elems per feat DMA (32 KB/partition chunk)
NBQ = B // BQ       # 4

_CACHE = {}


def _build_program():
    import concourse.bacc as bacc
    import concourse.bass as bass
    import concourse.tile as tile
    from concourse import mybir
    from concourse.masks import make_identity

    fp32 = mybir.dt.float32
    fp16 = mybir.dt.float16
    AF = mybir.ActivationFunctionType
    OP = mybir.AluOpType

    nc = bacc.Bacc("TRN2", target_bir_lowering=False, debug=False,
                   num_devices=NCORES)

    feat_d = nc.dram_tensor("feat", [TLOC, B, FEAT], fp32, kind="ExternalInput")
    r_d = nc.dram_tensor("r", [TLOC, B], fp32, kind="ExternalInput")
    w1_d = nc.dram_tensor("w1", [FEAT, HID], fp32, kind="ExternalInput")
    b1_d = nc.dram_tensor("b1", [HID], fp32, kind="ExternalInput")
    w2_d = nc.dram_tensor("w2", [HID], fp32, kind="ExternalInput")
    b2_d = nc.dram_tensor("b2", [1], fp32, kind="ExternalInput")
    z_d = nc.dram_tensor("z", [NH, 128, TLOC], fp32, kind="ExternalOutput")
    p_d = nc.dram_tensor("p", [NH, 128, TLOC], fp32, kind="ExternalOutput")

    with tile.TileContext(nc) as tc:
        with (
            tc.tile_pool(name="singles", bufs=1) as singles,
            tc.tile_pool(name="featin", bufs=3) as featin,
            tc.tile_pool(name="rin", bufs=2) as rin,
            tc.tile_pool(name="ftps", bufs=3, space="PSUM") as ftps,
            tc.tile_pool(name="hps", bufs=2, space="PSUM") as hps,
            tc.tile_pool(name="ftsb", bufs=3) as ftsb,
            tc.tile_pool(name="hwork", bufs=2) as hwork,
        ):
            # ---------------- constants ----------------
            ident = singles.tile([128, 128], fp16)
            make_identity(nc, ident)
            ident32 = singles.tile([128, 128], fp32)
            make_identity(nc, ident32)
            w1_sb = singles.tile([128, HID], fp16)
            nc.gpsimd.dma_start(w1_sb, w1_d[:, :])
            b1rep = singles.tile([128, 32, HID], fp32)
            nc.gpsimd.dma_start(
                b1rep, bass.AP(b1_d, 0, [[0, 128], [0, 32], [1, HID]]))
            w2rep = singles.tile([128, 32, HID], fp32)
            nc.gpsimd.dma_start(
                w2rep, bass.AP(w2_d, 0, [[0, 128], [0, 32], [1, HID]]))
            b2col = singles.tile([128, 1], fp32)
            nc.gpsimd.dma_start(b2col, bass.AP(b2_d, 0, [[0, 128], [1, 1]]))
            ones_sb = singles.tile([128, TLOC], fp32)
            nc.vector.memset(ones_sb, 1.0)

            # ---- r: load [t, b]; PE-transpose to rT [b, t] per half ----
            rT = [singles.tile([128, TLOC], fp32, tag=f"rT{h}", name=f"rT{h}")
                  for h in range(NH)]
            for tcnk in range(NTC):
                rload = rin.tile([128, B], fp32, tag="rload")
                nc.sync.dma_start(rload, r_d[tcnk * 128:(tcnk + 1) * 128, :])
                rps = ftps.tile([128, 4, 128], fp32, tag="ftp16")
                for h in range(NH):
                    nc.tensor.transpose(
                        rps[:, h, :], rload[:, h * 128:(h + 1) * 128], ident32)
                for h in range(NH):
                    nc.vector.tensor_copy(
                        rT[h][:, tcnk * 128:(tcnk + 1) * 128], rps[:, h, :])

            # per-tchunk alpha_pre accumulators [128 t, B]
            apre = [singles.tile([128, B], fp32, tag=f"apre{tc_}",
                                 name=f"apre{tc_}")
                    for tc_ in range(NTC)]
            # transposed alpha_pre [128 b, t] per half
            apreT = [singles.tile([128, TLOC], fp32, tag=f"apreT{h}",
                                  name=f"apreT{h}")
                     for h in range(NH)]

            # ---------------- main feat pipeline ----------------
            copy_parity = 0
            for tcnk in range(NTC):
                for bq in range(NBQ):
                    ft = featin.tile([128, BQ * FEAT], fp16, tag="ft")
                    nc.gpsimd.dma_start(
                        ft,
                        feat_d[tcnk * 128:(tcnk + 1) * 128,
                               bq * BQ:(bq + 1) * BQ, :].rearrange(
                                   "t b f -> t (b f)"))

                    for g in range(BQ // 32):  # 32-b half of this quarter
                        hbank = hps.tile([128, 32, HID], fp32, tag="hbank")
                        for q in range(0, 32, 4):
                            ftp = ftps.tile([128, 4, 128], fp16, tag="ftp16")
                            for s in range(4):
                                bl = g * 32 + q + s
                                nc.tensor.transpose(
                                    ftp[:, s, :],
                                    ft[:, bl * FEAT:(bl + 1) * FEAT], ident)
                            fts = ftsb.tile([128, 4, 128], fp16, tag="fts")
                            if copy_parity == 0:
                                nc.vector.tensor_copy(fts, ftp)
                            else:
                                nc.scalar.copy(fts, ftp)
                            copy_parity ^= 1
                            for s in range(4):
                                nc.tensor.matmul(hbank[:, q + s, :],
                                                 fts[:, s, :], w1_sb)

                        # drain this 32-b bank -> apre columns
                        b0 = bq * BQ + g * 32
                        hb = hwork.tile([128, 32, HID], fp32, tag="hb")
                        nc.vector.tensor_add(hb, hbank, b1rep)
                        hrelu = hwork.tile([128, 32, HID], fp32, tag="hrelu")
                        nc.scalar.activation(hrelu, hb, AF.Relu)
                        hw = hwork.tile([128, 32, HID], fp32, tag="hw")
                        nc.vector.tensor_mul(hw, hrelu, w2rep)
                        nc.vector.tensor_reduce(
                            apre[tcnk][:, b0:b0 + 32],
                            hw, axis=mybir.AxisListType.X, op=OP.add)

                # all of apre[tcnk] done: transpose [t, b] -> [b, t] halves
                aps = ftps.tile([128, 4, 128], fp32, tag="ftp16")
                for h in range(NH):
                    nc.tensor.transpose(
                        aps[:, h, :],
                        apre[tcnk][:, h * 128:(h + 1) * 128], ident32)
                for h in range(NH):
                    nc.scalar.copy(
                        apreT[h][:, tcnk * 128:(tcnk + 1) * 128], aps[:, h, :])

            # ---------------- alpha -> scans -> out ----------------
            for h in range(NH):
                alpha = singles.tile([128, TLOC], fp32, tag=f"alpha{h}")
                nc.scalar.activation(alpha, apreT[h], AF.Sigmoid, bias=b2col)
                nc.vector.tensor_scalar(alpha, alpha, 0.01, 0.99,
                                        op0=OP.max, op1=OP.min)
                A_sb = singles.tile([128, TLOC], fp32, tag=f"A{h}")
                nc.vector.tensor_scalar(A_sb, alpha, -1.0, 1.0,
                                        op0=OP.mult, op1=OP.add)
                Bv = singles.tile([128, TLOC], fp32, tag=f"Bv{h}")
                nc.vector.tensor_mul(Bv, alpha, rT[h])
                z_sb = singles.tile([128, TLOC], fp32, tag=f"z{h}")
                nc.vector.tensor_tensor_scan(z_sb, A_sb, Bv, 0.0,
                                             op0=OP.mult, op1=OP.add)
                p_sb = singles.tile([128, TLOC], fp32, tag=f"p{h}")
                nc.vector.tensor_tensor_scan(p_sb, A_sb, ones_sb, 1.0,
                                             op0=OP.mult, op1=OP.mult)
                nc.sync.dma_start(z_d[h], z_sb)
                nc.sync.dma_start(p_d[h], p_sb)

    nc.finalize()
    return nc


def _get_program():
    if "nc" not in _CACHE:
        _CACHE["nc"] = _build_program()
    return _CACHE["nc"]


def kernel(r, feat, W1, b1, W2, b2, _run_kwargs=None, _return_results=False):
    from concourse.bass_utils import run_bass_kernel_spmd

    r = np.asarray(r, dtype=np.float32)
    feat = np.asarray(feat, dtype=np.float32)
    W1 = np.asarray(W1, dtype=np.float32)
    b1 = np.asarray(b1, dtype=np.float32).reshape(HID)
    W2 = np.asarray(W2, dtype=np.float32).reshape(HID)
    b2 = np.asarray(b2, dtype=np.float32).reshape(1)

    nc = _get_program()
    in_maps = []
    for c in range(NCORES):
        in_maps.append({
            "feat": np.ascontiguousarray(feat[c * TLOC:(c + 1) * TLOC]),
            "r": np.ascontiguousarray(r[c * TLOC:(c + 1) * TLOC, :, 0]),
            "w1": W1, "b1": b1, "w2": W2, "b2": b2,
        })

    kw = _run_kwargs or {}
    res = run_bass_kernel_spmd(nc, in_maps, core_ids=list(range(NCORES)), **kw)

    # host stitch: y = z + P*carry per slab, carry chain across slabs
    # z/p layout: [h, p, t] with b = h*128 + p (contiguous halves)
    y = np.empty((T, B), dtype=np.float32)
    carry = r[0, :, 0].astype(np.float32)
    for c in range(NCORES):
        zc = res.results[c]["z"].reshape(B, TLOC).T
        pc = res.results[c]["p"].reshape(B, TLOC).T
        y_slab = zc + pc * carry[None, :]
        carry = y_slab[-1]
        y[c * TLOC:(c + 1) * TLOC] = y_slab
    out = y[:, :, None]
    if _return_results:
        return out, res
    return out


# revision 5
# speedup vs baseline: 1.4702x; 1.1825x over previous
"""EMA head kernel for Trainium2 (Bass/Tile), 8 NeuronCores.

Problem: alpha = clip(sigmoid(MLP(feat)), 0.01, 0.99) per (t, b);
         y[0] = r[0]; y[t] = (1-alpha[t])*y[t-1] + alpha[t]*r[t].

Sharding: time dim T=4096 split into 8 slabs of 512 (all B=256 per core).
Each core computes, for its slab, the local affine-scan pieces
    z[t] = A[t]*z[t-1] + Bv[t]   (z[-1] = 0),   A = 1-alpha, Bv = alpha*r
    P[t] = A[t]*P[t-1]           (P[-1] = 1)
and the host stitches slabs with   y = z + P * carry,  carry' = y[-1].
carry_0 = r[0] reproduces y[0] = r[0] exactly: a*r + (1-a)*r = r.

v3: feat is pre-cast to fp16 on the host (the MLP is computed in fp16
anyway), halving HBM read traffic, and loaded via HWDGE (sync) with the
TIME dim on partitions: tile [t=128, b=64 * f=128] so each partition
line is one 16 KB contiguous DRAM chunk (line-rate HBM).  Per batch
element: PE transpose [t,f] -> [f,t] (groups of 8 fill one 2 KB PSUM
bank), copy PSUM->SBUF (DVE/ACT alternating), matmul lhsT=ftT rhs=W1
-> h [t, 16] collected 32 b-slots per PSUM bank, then drain
+b1 (DVE, fused PSUM read) / relu (ACT) / *W2 + reduce (GPSIMD)
-> apre [t=128, b].  apre is PE-transposed back to [b=128, t] for
sigmoid/clip and the tensor_tensor_scan along t for z and P.
"""

import numpy as np

T, B, FEAT, HID = 4096, 256, 128, 16
NCORES = 8
TLOC = T // NCORES  # 512
NH = 2              # batch halves of 128 (contiguous: b = h*128 + p)
NTC = TLOC // 128   # 4 t-chunks of 128 partitions
BQ = 64             # batch elems per feat DMA (16 KB/partition chunk)
NBQ = B // BQ       # 4

_CACHE = {}


def _build_program():
    import concourse.bacc as bacc
    import concourse.bass as bass
    import concourse.tile as tile
    from concourse import mybir
    from concourse.masks import make_identity

    fp32 = mybir.dt.float32
    fp16 = mybir.dt.float16
    AF = mybir.ActivationFunctionType
    OP = mybir.AluOpType

    nc = bacc.Bacc("TRN2", target_bir_lowering=False, debug=False,
                   num_devices=NCORES)

    feat_d = nc.dram_tensor("feat", [TLOC, B, FEAT], fp16, kind="ExternalInput")
    r_d = nc.dram_tensor("r", [TLOC, B], fp32, kind="ExternalInput")
    w1_d = nc.dram_tensor("w1", [FEAT, HID], fp32, kind="ExternalInput")
    b1_d = nc.dram_tensor("b1", [HID], fp32, kind="ExternalInput")
    w2_d = nc.dram_tensor("w2", [HID], fp32, kind="ExternalInput")
    b2_d = nc.dram_tensor("b2", [1], fp32, kind="ExternalInput")
    z_d = nc.dram_tensor("z", [NH, 128, TLOC], fp32, kind="ExternalOutput")
    p_d = nc.dram_tensor("p", [NH, 128, TLOC], fp32, kind="ExternalOutput")

    with tile.TileContext(nc) as tc:
        with (
            tc.tile_pool(name="singles", bufs=1) as singles,
            tc.tile_pool(name="featin", bufs=3) as featin,
            tc.tile_pool(name="rin", bufs=2) as rin,
            tc.tile_pool(name="ftps", bufs=3, space="PSUM") as ftps,
            tc.tile_pool(name="hps", bufs=2, space="PSUM") as hps,
            tc.tile_pool(name="ftsb", bufs=3) as ftsb,
            tc.tile_pool(name="hwork", bufs=2) as hwork,
        ):
            # ---------------- constants ----------------
            ident = singles.tile([128, 128], fp16)
            make_identity(nc, ident)
            ident32 = singles.tile([128, 128], fp32)
            make_identity(nc, ident32)
            w1_sb = singles.tile([128, HID], fp16)
            nc.gpsimd.dma_start(w1_sb, w1_d[:, :])
            b1rep = singles.tile([128, 32, HID], fp32)
            nc.gpsimd.dma_start(
                b1rep, bass.AP(b1_d, 0, [[0, 128], [0, 32], [1, HID]]))
            w2rep = singles.tile([128, 32, HID], fp32)
            nc.gpsimd.dma_start(
                w2rep, bass.AP(w2_d, 0, [[0, 128], [0, 32], [1, HID]]))
            b2col = singles.tile([128, 1], fp32)
            nc.gpsimd.dma_start(b2col, bass.AP(b2_d, 0, [[0, 128], [1, 1]]))
            ones_sb = singles.tile([128, TLOC], fp32)
            nc.vector.memset(ones_sb, 1.0)

            # ---- r: load [t, b]; PE-transpose to rT [b, t] per half ----
            rT = [singles.tile([128, TLOC], fp32, tag=f"rT{h}", name=f"rT{h}")
                  for h in range(NH)]
            for tcnk in range(NTC):
                rload = rin.tile([128, B], fp32, tag="rload")
                nc.sync.dma_start(rload, r_d[tcnk * 128:(tcnk + 1) * 128, :])
                rps = ftps.tile([128, 4, 128], fp32, tag="rps")
                for h in range(NH):
                    nc.tensor.transpose(
                        rps[:, h, :], rload[:, h * 128:(h + 1) * 128], ident32)
                for h in range(NH):
                    nc.vector.tensor_copy(
                        rT[h][:, tcnk * 128:(tcnk + 1) * 128], rps[:, h, :])

            # per-tchunk alpha_pre accumulators [128 t, B]
            apre = [singles.tile([128, B], fp32, tag=f"apre{tc_}",
                                 name=f"apre{tc_}")
                    for tc_ in range(NTC)]
            # transposed alpha_pre [128 b, t] per half
            apreT = [singles.tile([128, TLOC], fp32, tag=f"apreT{h}",
                                  name=f"apreT{h}")
                     for h in range(NH)]

            # ---------------- main feat pipeline ----------------
            copy_parity = 0
            for tcnk in range(NTC):
                for bq in range(NBQ):
                    ft = featin.tile([128, BQ * FEAT], fp16, tag="ft")
                    nc.sync.dma_start(
                        ft,
                        feat_d[tcnk * 128:(tcnk + 1) * 128,
                               bq * BQ:(bq + 1) * BQ, :].rearrange(
                                   "t b f -> t (b f)"))

                    for g in range(BQ // 32):  # 32-b half of this quarter
                        hbank = hps.tile([128, 32, HID], fp32, tag="hbank")
                        for q in range(0, 32, 8):
                            ftp = ftps.tile([128, 8, 128], fp16, tag="ftp16")
                            for s in range(8):
                                bl = g * 32 + q + s
                                nc.tensor.transpose(
                                    ftp[:, s, :],
                                    ft[:, bl * FEAT:(bl + 1) * FEAT], ident)
                            fts = ftsb.tile([128, 8, 128], fp16, tag="fts")
                            if copy_parity == 0:
                                nc.vector.tensor_copy(fts, ftp)
                            else:
                                nc.scalar.copy(fts, ftp)
                            copy_parity ^= 1
                            for s in range(8):
                                nc.tensor.matmul(hbank[:, q + s, :],
                                                 fts[:, s, :], w1_sb)

                        # drain this 32-b bank -> apre columns
                        b0 = bq * BQ + g * 32
                        hb = hwork.tile([128, 32, HID], fp32, tag="hb")
                        nc.vector.tensor_add(hb, hbank, b1rep)
                        hrelu = hwork.tile([128, 32, HID], fp32, tag="hrelu")
                        nc.scalar.activation(hrelu, hb, AF.Relu)
                        hw = hwork.tile([128, 32, HID], fp32, tag="hw")
                        nc.gpsimd.tensor_mul(hw, hrelu, w2rep)
                        nc.vector.tensor_reduce(
                            apre[tcnk][:, b0:b0 + 32],
                            hw, axis=mybir.AxisListType.X, op=OP.add)

                # all of apre[tcnk] done: transpose [t, b] -> [b, t] halves
                aps = ftps.tile([128, 4, 128], fp32, tag="rps")
                for h in range(NH):
                    nc.tensor.transpose(
                        aps[:, h, :],
                        apre[tcnk][:, h * 128:(h + 1) * 128], ident32)
                for h in range(NH):
                    nc.scalar.copy(
                        apreT[h][:, tcnk * 128:(tcnk + 1) * 128], aps[:, h, :])

            # ---------------- alpha -> scans -> out ----------------
            for h in range(NH):
                alpha = singles.tile([128, TLOC], fp32, tag=f"alpha{h}")
                nc.scalar.activation(alpha, apreT[h], AF.Sigmoid, bias=b2col)
                nc.vector.tensor_scalar(alpha, alpha, 0.01, 0.99,
                                        op0=OP.max, op1=OP.min)
                A_sb = singles.tile([128, TLOC], fp32, tag=f"A{h}")
                nc.vector.tensor_scalar(A_sb, alpha, -1.0, 1.0,
                                        op0=OP.mult, op1=OP.add)
                Bv = singles.tile([128, TLOC], fp32, tag=f"Bv{h}")
                nc.vector.tensor_mul(Bv, alpha, rT[h])
                z_sb = singles.tile([128, TLOC], fp32, tag=f"z{h}")
                nc.vector.tensor_tensor_scan(z_sb, A_sb, Bv, 0.0,
                                             op0=OP.mult, op1=OP.add)
                p_sb = singles.tile([128, TLOC], fp32, tag=f"p{h}")
                nc.vector.tensor_tensor_scan(p_sb, A_sb, ones_sb, 1.0,
                                             op0=OP.mult, op1=OP.mult)
                nc.sync.dma_start(z_d[h], z_sb)
                nc.sync.dma_start(p_d[h], p_sb)

    nc.finalize()
    return nc


def _get_program():
    if "nc" not in _CACHE:
        _CACHE["nc"] = _build_program()
    return _CACHE["nc"]


def kernel(r, feat, W1, b1, W2, b2, _run_kwargs=None, _return_results=False):
    from concourse.bass_utils import run_bass_kernel_spmd

    r = np.asarray(r, dtype=np.float32)
    feat16 = np.asarray(feat, dtype=np.float16)
    W1 = np.asarray(W1, dtype=np.float32)
    b1 = np.asarray(b1, dtype=np.float32).reshape(HID)
    W2 = np.asarray(W2, dtype=np.float32).reshape(HID)
    b2 = np.asarray(b2, dtype=np.float32).reshape(1)

    nc = _get_program()
    in_maps = []
    for c in range(NCORES):
        in_maps.append({
            "feat": np.ascontiguousarray(feat16[c * TLOC:(c + 1) * TLOC]),
            "r": np.ascontiguousarray(r[c * TLOC:(c + 1) * TLOC, :, 0]),
            "w1": W1, "b1": b1, "w2": W2, "b2": b2,
        })

    kw = _run_kwargs or {}
    res = run_bass_kernel_spmd(nc, in_maps, core_ids=list(range(NCORES)), **kw)

    # host stitch: y = z + P*carry per slab, carry chain across slabs
    # z/p layout: [h, p, t] with b = h*128 + p (contiguous halves)
    y = np.empty((T, B), dtype=np.float32)
    carry = r[0, :, 0].astype(np.float32)
    for c in range(NCORES):
        zc = res.results[c]["z"].reshape(B, TLOC).T
        pc = res.results[c]["p"].reshape(B, TLOC).T
        y_slab = zc + pc * carry[None, :]
        carry = y_slab[-1]
        y[c * TLOC:(c + 1) * TLOC] = y_slab
    out = y[:, :, None]
    if _return_results:
        return out, res
    return out


# revision 8
# speedup vs baseline: 2.0699x; 1.4079x over previous
"""EMA head kernel for Trainium2 (Bass/Tile), 8 NeuronCores.

Problem: alpha = clip(sigmoid(MLP(feat)), 0.01, 0.99) per (t, b);
         y[0] = r[0]; y[t] = (1-alpha[t])*y[t-1] + alpha[t]*r[t].

Sharding: time dim T=4096 split into 8 slabs of 512 (all B=256 per core).
Each core computes, for its slab, the local affine-scan pieces
    z[t] = A[t]*z[t-1] + Bv[t]   (z[-1] = 0),   A = 1-alpha, Bv = alpha*r
    P[t] = A[t]*P[t-1]           (P[-1] = 1)
and the host stitches slabs with   y = z + P * carry,  carry' = y[-1].
carry_0 = r[0] reproduces y[0] = r[0] exactly: a*r + (1-a)*r = r.

v3: feat is pre-cast to fp16 on the host (the MLP is computed in fp16
anyway), halving HBM read traffic, and loaded via HWDGE (sync) with the
TIME dim on partitions: tile [t=128, b=64 * f=128] so each partition
line is one 16 KB contiguous DRAM chunk (line-rate HBM).  Per batch
element: PE transpose [t,f] -> [f,t] (groups of 8 fill one 2 KB PSUM
bank), copy PSUM->SBUF (DVE/ACT alternating), matmul lhsT=ftT rhs=W1
-> h [t, 16] collected 32 b-slots per PSUM bank, then drain
+b1 (DVE, fused PSUM read) / relu (ACT) / *W2 + reduce (GPSIMD)
-> apre [t=128, b].  apre is PE-transposed back to [b=128, t] for
sigmoid/clip and the tensor_tensor_scan along t for z and P.
"""

import numpy as np

T, B, FEAT, HID = 4096, 256, 128, 16
NCORES = 8
TLOC = T // NCORES  # 512
NH = 2              # batch halves of 128 (contiguous: b = h*128 + p)
NTC = TLOC // 128   # 4 t-chunks of 128 partitions
BQ = 64             # batch elems per feat DMA (16 KB/partition chunk)
NBQ = B // BQ       # 4

_CACHE = {}


def _build_program():
    import concourse.bacc as bacc
    import concourse.bass as bass
    import concourse.tile as tile
    from concourse import mybir
    from concourse.masks import make_identity

    fp32 = mybir.dt.float32
    fp16 = mybir.dt.float16
    AF = mybir.ActivationFunctionType
    OP = mybir.AluOpType

    nc = bacc.Bacc("TRN2", target_bir_lowering=False, debug=False,
                   num_devices=NCORES)

    feat_d = nc.dram_tensor("feat", [TLOC, B, FEAT], fp16, kind="ExternalInput")
    r_d = nc.dram_tensor("r", [TLOC, B], fp32, kind="ExternalInput")
    w1_d = nc.dram_tensor("w1", [FEAT, HID], fp16, kind="ExternalInput")
    b1_d = nc.dram_tensor("b1rep", [128, 32, HID], fp32, kind="ExternalInput")
    w2_d = nc.dram_tensor("w2rep", [128, 32, HID], fp32, kind="ExternalInput")
    b2_d = nc.dram_tensor("b2col", [128, 1], fp32, kind="ExternalInput")
    z_d = nc.dram_tensor("z", [NH, 128, TLOC], fp32, kind="ExternalOutput")
    p_d = nc.dram_tensor("p", [NH, 128, TLOC], fp32, kind="ExternalOutput")

    with tile.TileContext(nc) as tc:
        with (
            tc.tile_pool(name="singles", bufs=1) as singles,
            tc.tile_pool(name="featin", bufs=3) as featin,
            tc.tile_pool(name="rin", bufs=2) as rin,
            tc.tile_pool(name="ftps", bufs=3, space="PSUM") as ftps,
            tc.tile_pool(name="hps", bufs=2, space="PSUM") as hps,
            tc.tile_pool(name="ftsb", bufs=3) as ftsb,
            tc.tile_pool(name="hwork", bufs=2) as hwork,
        ):
            # ---------------- constants ----------------
            ident = singles.tile([128, 128], fp16)
            make_identity(nc, ident)
            ident32 = singles.tile([128, 128], fp32)
            make_identity(nc, ident32)
            w1_sb = singles.tile([128, HID], fp16)
            nc.sync.dma_start(w1_sb, w1_d[:, :])
            b1rep = singles.tile([128, 32, HID], fp32)
            nc.sync.dma_start(b1rep, b1_d[:, :, :])
            w2rep = singles.tile([128, 32, HID], fp32)
            nc.sync.dma_start(w2rep, w2_d[:, :, :])
            b2col = singles.tile([128, 1], fp32)
            nc.sync.dma_start(b2col, b2_d[:, :])
            ones_sb = singles.tile([128, TLOC], fp32)
            nc.vector.memset(ones_sb, 1.0)

            # ---- r: load [t, b]; PE-transpose to rT [b, t] per half ----
            rT = [singles.tile([128, TLOC], fp32, tag=f"rT{h}", name=f"rT{h}")
                  for h in range(NH)]
            for tcnk in range(NTC):
                rload = rin.tile([128, B], fp32, tag="rload")
                nc.sync.dma_start(rload, r_d[tcnk * 128:(tcnk + 1) * 128, :])
                rps = ftps.tile([128, 4, 128], fp32, tag="rps")
                for h in range(NH):
                    nc.tensor.transpose(
                        rps[:, h, :], rload[:, h * 128:(h + 1) * 128], ident32)
                for h in range(NH):
                    nc.vector.tensor_copy(
                        rT[h][:, tcnk * 128:(tcnk + 1) * 128], rps[:, h, :])

            # per-tchunk alpha_pre accumulators [128 t, B]
            apre = [singles.tile([128, B], fp32, tag=f"apre{tc_}",
                                 name=f"apre{tc_}")
                    for tc_ in range(NTC)]
            # transposed alpha_pre [128 b, t] per half
            apreT = [singles.tile([128, TLOC], fp32, tag=f"apreT{h}",
                                  name=f"apreT{h}")
                     for h in range(NH)]

            # ---------------- main feat pipeline ----------------
            copy_parity = 0
            for tcnk in range(NTC):
                for bq in range(NBQ):
                    ft = featin.tile([128, BQ * FEAT], fp16, tag="ft")
                    nc.sync.dma_start(
                        ft,
                        feat_d[tcnk * 128:(tcnk + 1) * 128,
                               bq * BQ:(bq + 1) * BQ, :].rearrange(
                                   "t b f -> t (b f)"))

                    for g in range(BQ // 32):  # 32-b half of this quarter
                        hbank = hps.tile([128, 32, HID], fp32, tag="hbank")
                        for q in range(0, 32, 8):
                            ftp = ftps.tile([128, 8, 128], fp16, tag="ftp16")
                            for s in range(8):
                                bl = g * 32 + q + s
                                nc.tensor.transpose(
                                    ftp[:, s, :],
                                    ft[:, bl * FEAT:(bl + 1) * FEAT], ident)
                            fts = ftsb.tile([128, 8, 128], fp16, tag="fts")
                            if copy_parity == 0:
                                nc.vector.tensor_copy(fts, ftp)
                            else:
                                nc.scalar.copy(fts, ftp)
                            copy_parity ^= 1
                            for s in range(8):
                                nc.tensor.matmul(hbank[:, q + s, :],
                                                 fts[:, s, :], w1_sb)

                        # drain this 32-b bank -> apre columns
                        b0 = bq * BQ + g * 32
                        hb = hwork.tile([128, 32, HID], fp32, tag="hb")
                        nc.vector.tensor_add(hb, hbank, b1rep)
                        hrelu = hwork.tile([128, 32, HID], fp32, tag="hrelu")
                        nc.scalar.activation(hrelu, hb, AF.Relu)
                        hw = hwork.tile([128, 32, HID], fp32, tag="hw")
                        nc.gpsimd.tensor_mul(hw, hrelu, w2rep)
                        nc.vector.tensor_reduce(
                            apre[tcnk][:, b0:b0 + 32],
                            hw, axis=mybir.AxisListType.X, op=OP.add)

                # all of apre[tcnk] done: transpose [t, b] -> [b, t] halves
                aps = ftps.tile([128, 4, 128], fp32, tag="rps")
                for h in range(NH):
                    nc.tensor.transpose(
                        aps[:, h, :],
                        apre[tcnk][:, h * 128:(h + 1) * 128], ident32)
                for h in range(NH):
                    nc.scalar.copy(
                        apreT[h][:, tcnk * 128:(tcnk + 1) * 128], aps[:, h, :])

            # ---------------- alpha -> scans -> out ----------------
            for h in range(NH):
                alpha = singles.tile([128, TLOC], fp32, tag=f"alpha{h}")
                nc.scalar.activation(alpha, apreT[h], AF.Sigmoid, bias=b2col)
                nc.vector.tensor_scalar(alpha, alpha, 0.01, 0.99,
                                        op0=OP.max, op1=OP.min)
                A_sb = singles.tile([128, TLOC], fp32, tag=f"A{h}")
                nc.vector.tensor_scalar(A_sb, alpha, -1.0, 1.0,
                                        op0=OP.mult, op1=OP.add)
                Bv = singles.tile([128, TLOC], fp32, tag=f"Bv{h}")
                nc.vector.tensor_mul(Bv, alpha, rT[h])
                z_sb = singles.tile([128, TLOC], fp32, tag=f"z{h}")
                nc.vector.tensor_tensor_scan(z_sb, A_sb, Bv, 0.0,
                                             op0=OP.mult, op1=OP.add)
                p_sb = singles.tile([128, TLOC], fp32, tag=f"p{h}")
                nc.vector.tensor_tensor_scan(p_sb, A_sb, ones_sb, 1.0,
                                             op0=OP.mult, op1=OP.mult)
                nc.sync.dma_start(z_d[h], z_sb)
                nc.sync.dma_start(p_d[h], p_sb)

    nc.finalize()
    return nc


def _get_program():
    if "nc" not in _CACHE:
        _CACHE["nc"] = _build_program()
    return _CACHE["nc"]


def kernel(r, feat, W1, b1, W2, b2, _run_kwargs=None, _return_results=False):
    from concourse.bass_utils import run_bass_kernel_spmd

    r = np.asarray(r, dtype=np.float32)
    feat16 = np.asarray(feat, dtype=np.float16)
    W1 = np.asarray(W1, dtype=np.float16)
    b1rep = np.ascontiguousarray(np.broadcast_to(
        np.asarray(b1, dtype=np.float32).reshape(1, 1, HID), (128, 32, HID)))
    w2rep = np.ascontiguousarray(np.broadcast_to(
        np.asarray(W2, dtype=np.float32).reshape(1, 1, HID), (128, 32, HID)))
    b2col = np.ascontiguousarray(np.broadcast_to(
        np.asarray(b2, dtype=np.float32).reshape(1, 1), (128, 1)))

    nc = _get_program()
    in_maps = []
    for c in range(NCORES):
        in_maps.append({
            "feat": np.ascontiguousarray(feat16[c * TLOC:(c + 1) * TLOC]),
            "r": np.ascontiguousarray(r[c * TLOC:(c + 1) * TLOC, :, 0]),
            "w1": W1, "b1rep": b1rep, "w2rep": w2rep, "b2col": b2col,
        })

    kw = _run_kwargs or {}
    res = run_bass_kernel_spmd(nc, in_maps, core_ids=list(range(NCORES)), **kw)

    # host stitch: y = z + P*carry per slab, carry chain across slabs
    # z/p layout: [h, p, t] with b = h*128 + p (contiguous halves)
    y = np.empty((T, B), dtype=np.float32)
    carry = r[0, :, 0].astype(np.float32)
    for c in range(NCORES):
        zc = res.results[c]["z"].reshape(B, TLOC).T
        pc = res.results[c]["p"].reshape(B, TLOC).T
        y_slab = zc + pc * carry[None, :]
        carry = y_slab[-1]
        y[c * TLOC:(c + 1) * TLOC] = y_slab
    out = y[:, :, None]
    if _return_results:
        return out, res
    return out
